# revision 9
# baseline (speedup 1.0000x reference)
"""Trainium2 Bass kernel for 7x7 sliding-window self-similarity attention.

out[b,c,h,w] = sum_j softmax_j(x[h,w] * x[h+dh,w+dw]) * x[h+dh,w+dw]
over the 7x7 neighborhood (zero padding, pad=3).

Sharding: B*C = 256 independent 128x128 images, 32 images per core on 8
NeuronCores (pure data parallel, no collectives).

Per-core layout: partition p = rowblock(0..3)*32 + image(0..31); each
partition holds a 44-row x 140-col zero-padded fp32 slab (32-row block +
6-row halo, 128 cols + 6-col pad) so every window shift is a free-dim view.
The host pre-builds the slabs (halo duplication) so the kernel does one
contiguous [128, 6160] DMA in and one [128, 4096] DMA out.

Score symmetry: e_{-d}[i] == e_d[i-d], so only 25 canonical score/exp tiles
are computed (on a 38x134 extended region); the mirrored 24 are views.
"""

import numpy as np

import concourse.bacc as bacc
import concourse.bass as bass  # noqa: F401  (AluOpType re-exports etc.)
import concourse.tile as tile
from concourse import mybir
from concourse.bass_utils import run_bass_kernel_spmd

N_CORES = 8
F32 = mybir.dt.float32
MULT = mybir.AluOpType.mult
ADD = mybir.AluOpType.add

B, C, H, W = 4, 64, 128, 128
N_IMG_TOTAL = B * C          # 256 independent images
IMG_PER_CORE = N_IMG_TOTAL // N_CORES  # 32
RB_N = 4                     # rowblocks per image
PAD = 6                      # host-side zero pad on each spatial side


def canonical_offsets():
    """(0,0) plus one representative of each +-delta pair: 25 total."""
    canon = [(0, 0)]
    canon += [(0, dj) for dj in range(1, 4)]
    canon += [(di, dj) for di in range(1, 4) for dj in range(-3, 4)]
    return canon


def build_nc(n_img=IMG_PER_CORE, h=H, w=W):
    """Build the single-core Bass program (SPMD across 8 cores)."""
    br = h // RB_N               # rows per block (32)
    wp = w + 2 * PAD             # 140
    slab = br + 2 * PAD          # 44 stored rows per partition
    P = n_img * RB_N             # partitions used (128)
    er, ec = br + 6, w + 6       # extended (score/exp) region 38 x 134

    nc = bacc.Bacc("TRN2", target_bir_lowering=False, debug=False)
    x_in = nc.dram_tensor("x", [P, slab * wp], F32, kind="ExternalInput")
    y_out = nc.dram_tensor("y", [P, br * w], F32, kind="ExternalOutput")

    with tile.TileContext(nc) as tc:
        with (
            tc.tile_pool(name="big", bufs=1) as big,
            tc.tile_pool(name="se", bufs=2) as sepool,
            tc.tile_pool(name="mm", bufs=2) as mpool,
        ):
            x = big.tile([P, slab, wp], F32, tag="x")
            acc = big.tile([P, br * w], F32, tag="acc")
            sum_e = big.tile([P, br * w], F32, tag="sum")
            out_t = big.tile([P, br * w], F32, tag="out")

            nc.sync.dma_start(
                out=x[:].rearrange("p a b -> p (a b)"), in_=x_in[:]
            )

            acc3 = acc[:].rearrange("p (a b) -> p a b", a=br)
            sum3 = sum_e[:].rearrange("p (a b) -> p a b", a=br)

            for idx, (di, dj) in enumerate(canonical_offsets()):
                s = sepool.tile([P, er, ec], F32, tag="s")
                e = sepool.tile([P, er, ec], F32, tag="e")

                # scores on the extended region: s = x * shift(x, +d)
                nc.vector.tensor_tensor(
                    out=s[:],
                    in0=x[:, 3:3 + er, 3:3 + ec],
                    in1=x[:, 3 + di:3 + er + di, 3 + dj:3 + ec + dj],
                    op=MULT,
                )
                nc.scalar.activation(
                    out=e[:], in_=s[:], func=mybir.ActivationFunctionType.Exp
                )

                e0 = e[:, 3:3 + br, 3:3 + w]          # e_d at output pixels
                xq = x[:, 6 + di:6 + br + di, 6 + dj:6 + w + dj]
                if idx == 0:
                    # first term initializes the accumulators
                    nc.vector.tensor_tensor(out=acc3, in0=e0, in1=xq, op=MULT)
                    nc.gpsimd.tensor_copy(sum3, e0)
                    continue

                m = mpool.tile([P, br * w], F32, tag="m")
                m3 = m[:].rearrange("p (a b) -> p a b", a=br)
                nc.vector.tensor_tensor(out=m3, in0=e0, in1=xq, op=MULT)
                nc.vector.tensor_tensor(out=acc[:], in0=acc[:], in1=m[:], op=ADD)
                nc.gpsimd.tensor_tensor(out=sum3, in0=sum3, in1=e0, op=ADD)

                # mirrored neighbor -d via shifted views of the same e tile
                em = e[:, 3 - di:3 + br - di, 3 - dj:3 + w - dj]
                xm = x[:, 6 - di:6 + br - di, 6 - dj:6 + w - dj]
                m2 = mpool.tile([P, br * w], F32, tag="m")
                m23 = m2[:].rearrange("p (a b) -> p a b", a=br)
                nc.vector.tensor_tensor(out=m23, in0=em, in1=xm, op=MULT)
                nc.vector.tensor_tensor(out=acc[:], in0=acc[:], in1=m2[:], op=ADD)
                nc.gpsimd.tensor_tensor(out=sum3, in0=sum3, in1=em, op=ADD)

            r = mpool.tile([P, br * w], F32, tag="m")
            scr = mpool.tile([P, br * w], F32, tag="m")
            nc.vector.reciprocal_approx_accurate(
                out=r[:], in_=sum_e[:], scratch=scr[:]
            )
            nc.vector.tensor_tensor(out=out_t[:], in0=acc[:], in1=r[:], op=MULT)

            nc.sync.dma_start(out=y_out[:], in_=out_t[:])
    nc.compile()
    return nc


_NC_CACHE = {}


def _get_nc():
    if "nc" not in _NC_CACHE:
        _NC_CACHE["nc"] = build_nc()
    return _NC_CACHE["nc"]


def make_slabs(imgs, h=H, w=W):
    """[n,h,w] fp32 -> [n*4, 44*140] slab layout (p = rb*n + img)."""
    n = imgs.shape[0]
    br = h // RB_N
    slab = br + 2 * PAD
    xp = np.pad(imgs, ((0, 0), (PAD, PAD), (PAD, PAD)))
    rows = (np.arange(RB_N) * br)[:, None] + np.arange(slab)  # [4, 44]
    sl = xp[:, rows, :]                 # [n, 4, 44, wp]
    sl = sl.transpose(1, 0, 2, 3)       # [4, n, 44, wp]
    return np.ascontiguousarray(sl.reshape(RB_N * n, -1))


def unslab_out(y, n_img, h=H, w=W):
    """[n*4, br*w] -> [n, h, w]."""
    br = h // RB_N
    y = y.reshape(RB_N, n_img, br, w).transpose(1, 0, 2, 3)
    return np.ascontiguousarray(y.reshape(n_img, h, w))


def run(x, **spmd_kwargs):
    """Run on 8 cores; returns (full output, BassKernelResults)."""
    nc = _get_nc()
    imgs = np.ascontiguousarray(np.asarray(x).reshape(N_IMG_TOTAL, H, W))
    imgs = imgs.astype(np.float32, copy=False)
    in_maps = [
        {"x": make_slabs(imgs[i * IMG_PER_CORE:(i + 1) * IMG_PER_CORE])}
        for i in range(N_CORES)
    ]
    res = run_bass_kernel_spmd(nc, in_maps, core_ids=list(range(N_CORES)),
                               **spmd_kwargs)
    out = np.concatenate(
        [unslab_out(res.results[i]["y"], IMG_PER_CORE) for i in range(N_CORES)],
        axis=0,
    )
    return out.reshape(B, C, H, W).astype(np.float32, copy=False), res


def kernel(x):
    out, _ = run(x)
    return out
